# revision 4
# baseline (speedup 1.0000x reference)
"""Trainium2 Bass kernel for nn_AttentionBlock (GroupNorm + 1x1-conv QKV +
full self-attention over N=HW=4096 + output projection + residual).

Distribution: data-parallel over batch B=8, one batch element per NeuronCore.

Per-core layout / algorithm (C=128 channels on SBUF partitions, N=4096 free):
  1. GroupNorm stats via two ACT passes (Square + Identity, both with
     accum_out row-sums), cross-partition group combine via tiny indicator
     matmuls on the PE.
  2. hn = a_c * x + b_c  (ACT + DVE, output bf16).
  3. Q, K in natural [c, n] layout (lhsT = host-pretransposed weights, bf16
     so the FWL fast-weight-load path engages); V^T in [n, c] tile-major
     layout (lhsT = hn tiles).
  4. Main loop (2 halves x 32 j-tiles), software-pipelined so the PE never
     waits on its own iteration's exp: emit S(j) -> exp(j) -> O(j-1).
     exp is split across engines: 3 of 4 j-tiles on ACT (-> bf16 P), every
     4th on DVE via the Schraudolph bit-trick in bf16 bit-space:
     p = bitcast_bf16(int16(A*s + B)), A = 2^7/ln2 (+-3.3% rel err on the
     attention weights, which post-softmax-averaging is ~1e-3 on the
     output).  Denominator partials in acc (bf16, DVE 2x adds), reduced by
     a ones^T matmul at the end.
  5. Tail (rowsum -> reciprocal -> broadcast -> O-normalize -> projection ->
     residual) for the first half is interleaved into the second half's
     main loop, reusing the S PSUM tags, so it hides under main-loop work.

Bias algebra: b_k is dropped entirely -- k_j = Wk hn_j + b_k adds q_i.b_k to
every logit of query i, a per-i constant that cancels exactly in softmax.
b_q folded into the Q PSUM->SBUF copy; b_v folded into
b_eff = b_proj + w_proj @ b_v (host precompute, exact).  The attention scale
C^-0.5 is folded into w_q/b_q on the host (exact reparameterization).
"""

import numpy as np

B, C, H, W = 8, 128, 64, 64
HW = H * W                      # 4096
GROUPS = 8
GSIZE = C // GROUPS             # 16
EPS = 1e-5
NJ = HW // 128                  # 32 j-tiles
IBLK = 512
NIB = HW // IBLK                # 8 i-blocks
NHALF = 2
HWID = HW // NHALF              # 2048
SCALE = float(C) ** -0.5
# Schraudolph fast-exp in bf16 bit-space: p = bitcast_bf16(int16(A*s + B));
# B tuned for truncating f32->i16 conversion, minimax rel err ~3.3e-2 on
# s in [-8, 8].  int16 IS the bf16 bit pattern, so DVE-exp'd tiles are
# bf16 like the ACT ones -- one uniform pipeline.
EXP_A = float((1 << 7)) / float(np.log(2.0))
EXP_B = float(127 * (1 << 7)) - 5.10

_CACHE = {}


def _build():
    from contextlib import ExitStack

    import concourse.bacc as bacc
    import concourse.tile as tile
    from concourse import mybir

    f32 = mybir.dt.float32
    bf16 = mybir.dt.bfloat16
    i16 = mybir.dt.int16
    AF = mybir.ActivationFunctionType

    nc = bacc.Bacc("TRN2", target_bir_lowering=False, debug=False)

    x_in = nc.dram_tensor("x", [C, HW], f32, kind="ExternalInput")
    gamma_in = nc.dram_tensor("gamma", [C, 1], f32, kind="ExternalInput")
    beta_in = nc.dram_tensor("beta", [C, 1], f32, kind="ExternalInput")
    bq_in = nc.dram_tensor("bq", [C, 1], f32, kind="ExternalInput")
    beff_in = nc.dram_tensor("beff", [C, 1], f32, kind="ExternalInput")
    wq_in = nc.dram_tensor("wqT", [C, C], f32, kind="ExternalInput")
    wk_in = nc.dram_tensor("wkT", [C, C], f32, kind="ExternalInput")
    wv_in = nc.dram_tensor("wvT", [C, C], f32, kind="ExternalInput")
    wp_in = nc.dram_tensor("wpT", [C, C], f32, kind="ExternalInput")
    ig_in = nc.dram_tensor("ig", [C, GROUPS], f32, kind="ExternalInput")
    igt_in = nc.dram_tensor("igt", [GROUPS, C], f32, kind="ExternalInput")
    out_dram = nc.dram_tensor("out", [C, HW], f32, kind="ExternalOutput")

    DVE_EXP = 4  # every DVE_EXP-th j-tile exps on DVE instead of ACT

    with tile.TileContext(nc) as tc, ExitStack() as ctx, \
         nc.allow_low_precision(reason="bf16 attention pipeline + fast-exp; "
                                "error budget audited vs the 2e-2 gate"):
        const = ctx.enter_context(tc.tile_pool(name="const", bufs=1))
        big = ctx.enter_context(tc.tile_pool(name="big", bufs=1))
        stats = ctx.enter_context(tc.tile_pool(name="stats", bufs=1))
        ptpool = ctx.enter_context(tc.tile_pool(name="pt", bufs=3))
        row = ctx.enter_context(tc.tile_pool(name="row", bufs=1))
        stg = ctx.enter_context(tc.tile_pool(name="stage", bufs=3))

        # ---------------- load x first (sync queue), consts on the ACT
        # HWDGE queue so they don't serialize behind/ahead of x -------------
        NCH = 4
        CHW = HW // NCH
        x_sb = big.tile([C, HW], f32, tag="x")
        for ch in range(NCH):
            sl = slice(ch * CHW, (ch + 1) * CHW)
            eng = nc.sync if ch % 2 == 0 else nc.scalar
            eng.dma_start(x_sb[:, sl], x_in[:, sl])

        def cload(t_in, shape, tag):
            t = const.tile(shape, f32, tag=tag)
            nc.sync.dma_start(t[:], t_in[:])
            return t

        gamma = cload(gamma_in, [C, 1], "c_gamma")
        beta = cload(beta_in, [C, 1], "c_beta")
        bq = cload(bq_in, [C, 1], "c_bq")
        beff = cload(beff_in, [C, 1], "c_beff")
        ig = cload(ig_in, [C, GROUPS], "c_ig")
        igt = cload(igt_in, [GROUPS, C], "c_igt")
        wq_f = cload(wq_in, [C, C], "c_wq_f")
        wk_f = cload(wk_in, [C, C], "c_wk_f")
        wv_f = cload(wv_in, [C, C], "c_wv_f")
        wp_f = cload(wp_in, [C, C], "c_wp_f")

        wq = const.tile([C, C], bf16)
        nc.vector.tensor_copy(wq[:], wq_f[:])
        wk = const.tile([C, C], bf16)
        nc.vector.tensor_copy(wk[:], wk_f[:])
        wv = const.tile([C, C], bf16)
        nc.vector.tensor_copy(wv[:], wv_f[:])
        wp = const.tile([C, C], bf16)
        nc.vector.tensor_copy(wp[:], wp_f[:])

        ones_c = const.tile([C, 1], bf16)
        nc.vector.memset(ones_c[:], 1.0)
        ones_r = const.tile([1, C], bf16)
        nc.vector.memset(ones_r[:], 1.0)
        eps_t = const.tile([GROUPS, 1], f32)
        nc.vector.memset(eps_t[:], EPS)
        magic_t = const.tile([GROUPS, 1], mybir.dt.uint32)
        nc.vector.memset(magic_t[:], 0x5F3759DF)
        c15_t = const.tile([GROUPS, 1], f32)
        nc.vector.memset(c15_t[:], 1.5)

        # ---------------- groupnorm stats (split across DVE and ACT) ----
        st2 = stats.tile([C, 2], f32)
        s2p = stats.tile([C, NCH], f32)
        s1p = stats.tile([C, NCH], f32)
        adum = stats.tile([C, CHW], f32)
        for ch in range(NCH):  # x^2 sums on ACT, x sums on DVE
            sl = slice(ch * CHW, (ch + 1) * CHW)
            nc.scalar.activation(
                adum[:], x_sb[:, sl], AF.Square, accum_out=s2p[:, ch:ch + 1]
            )
            nc.vector.reduce_sum(
                s1p[:, ch:ch + 1], x_sb[:, sl], axis=mybir.AxisListType.X
            )
        warm = stats.tile([GROUPS, 1], f32)
        nc.scalar.activation(warm[:], eps_t[:], AF.Exp)
        nc.vector.reduce_sum(st2[:, 1:2], s2p[:], axis=mybir.AxisListType.X)
        nc.vector.reduce_sum(st2[:, 0:1], s1p[:], axis=mybir.AxisListType.X)

        # PSUM layout for the whole kernel body: two S tiles (2 banks each,
        # independently released) + one O accumulator (4 banks). The QKV
        # rounds, groupnorm matmuls AND the softmax/projection tail all
        # borrow the S slots so there is no pool barrier anywhere.
        acc = big.tile([C, HW], bf16, tag="acc")
        o_sb = big.tile([C, HW], bf16, tag="o")
        recip = row.tile([1, HW], bf16)
        rs128 = stats.tile([C, HW // C], f32)  # [128, 32]
        rc128 = stats.tile([C, HW // C], bf16)
        out_sb = big.tile([C, HW], f32, tag="scratch")
        HQ = HWID // 2  # 1024
        with tc.tile_pool(name="ps_s", bufs=1, space="PSUM") as ps_s, \
             tc.tile_pool(name="ps_o", bufs=1, space="PSUM") as ps_o:
            gs_ps = ps_s.tile([GROUPS, 2], f32, tag="s0")
            nc.tensor.matmul(gs_ps[:], ig[:], st2[:], start=True, stop=True)
            gstats = stats.tile([GROUPS, 2], f32)
            nc.vector.tensor_copy(gstats[:], gs_ps[:])
            inv_n = 1.0 / float(GSIZE * HW)
            gmean = stats.tile([GROUPS, 1], f32)
            nc.vector.tensor_scalar_mul(gmean[:], gstats[:, 0:1], inv_n)
            gm2 = stats.tile([GROUPS, 1], f32)
            nc.vector.tensor_scalar_mul(gm2[:], gstats[:, 1:2], inv_n)
            gmsq = stats.tile([GROUPS, 1], f32)
            nc.vector.tensor_mul(gmsq[:], gmean[:], gmean[:])
            gvar = stats.tile([GROUPS, 1], f32)
            nc.vector.tensor_sub(gvar[:], gm2[:], gmsq[:])
            gve = stats.tile([GROUPS, 1], f32)
            nc.vector.tensor_scalar(
                gve[:], gvar[:], eps_t[:], None, mybir.AluOpType.add
            )
            # rstd = rsqrt(var+eps): quake initial guess + Newton steps (DVE
            # only -- ACT Sqrt/Ln would each force a ~1.3us table-set swap)
            u32 = mybir.dt.uint32
            gu = stats.tile([GROUPS, 1], u32)
            nc.vector.tensor_scalar(
                gu[:], gve[:].bitcast(u32), 1, None,
                mybir.AluOpType.logical_shift_right,
            )
            nc.vector.tensor_sub(gu[:], magic_t[:], gu[:])
            gy = stats.tile([GROUPS, 1], f32)
            nc.vector.tensor_copy(gy[:], gu[:].bitcast(f32))
            gh = stats.tile([GROUPS, 1], f32)
            nc.vector.tensor_scalar_mul(gh[:], gve[:], 0.5)
            gt = stats.tile([GROUPS, 1], f32)
            for _ in range(2):
                nc.vector.tensor_mul(gt[:], gy[:], gy[:])
                nc.vector.tensor_mul(gt[:], gt[:], gh[:])
                nc.vector.tensor_sub(gt[:], c15_t[:], gt[:])
                nc.vector.tensor_mul(gy[:], gy[:], gt[:])
            gmr = stats.tile([GROUPS, 2], f32)
            nc.vector.tensor_copy(gmr[:, 1:2], gy[:])
            nc.vector.tensor_copy(gmr[:, 0:1], gmean[:])

            bc_ps = ps_s.tile([C, 2], f32, tag="s1")
            nc.tensor.matmul(bc_ps[:], igt[:], gmr[:], start=True, stop=True)
            a_c = stats.tile([C, 1], f32)
            b_c = stats.tile([C, 1], f32)
            tmc = stats.tile([C, 1], f32)
            nc.vector.tensor_scalar_mul(a_c[:], gamma[:], bc_ps[:, 1:2])
            nc.vector.tensor_scalar_mul(tmc[:], a_c[:], bc_ps[:, 0:1])
            nc.vector.tensor_sub(b_c[:], beta[:], tmc[:])

            hn = big.tile([C, HW], bf16, tag="hn")
            q_r = big.tile([C, HW], bf16, tag="q")
            k_r = big.tile([C, HW], bf16, tag="k")
            vt = big.tile([C, NJ, C], bf16, tag="vt")

            def emit_hn(h, engine):
                hs = slice(h * HWID, (h + 1) * HWID)
                if engine == "act":
                    nc.scalar.activation(
                        hn[:, hs], x_sb[:, hs], AF.Identity, bias=b_c[:], scale=a_c[:]
                    )
                else:
                    nc.vector.tensor_scalar(
                        hn[:, hs], x_sb[:, hs], a_c[:], b_c[:],
                        mybir.AluOpType.mult, mybir.AluOpType.add,
                    )

            def emit_k_round(h, r):  # r in 0..1, [C, HQ] rounds
                kp = ps_s.tile([C, HQ], f32, tag=f"s{r % 2}")
                for kk in range(2):
                    off = h * HWID + r * HQ + kk * IBLK
                    nc.tensor.matmul(
                        kp[:, kk * IBLK:(kk + 1) * IBLK], wk[:],
                        hn[:, off:off + IBLK], start=True, stop=True,
                    )
                # b_k dropped: a per-query constant in the logits, cancels in
                # softmax exactly.
                nc.vector.tensor_copy(
                    k_r[:, h * HWID + r * HQ:h * HWID + (r + 1) * HQ], kp[:]
                )

            def emit_q_round(h, r):
                qp = ps_s.tile([C, HQ], f32, tag=f"s{r % 2}")
                for kk in range(2):
                    off = h * HWID + r * HQ + kk * IBLK
                    nc.tensor.matmul(
                        qp[:, kk * IBLK:(kk + 1) * IBLK], wq[:],
                        hn[:, off:off + IBLK], start=True, stop=True,
                    )
                nc.vector.tensor_scalar(
                    q_r[:, h * HWID + r * HQ:h * HWID + (r + 1) * HQ], qp[:],
                    bq[:], None, mybir.AluOpType.add,
                )

            def emit_v_round(h, r):  # r in 0..3, 4 n-tiles per round
                vp = ps_s.tile([C, 4, C], f32, tag=f"s{r % 2}")
                for t in range(4):
                    nt = h * 16 + r * 4 + t
                    nc.tensor.matmul(
                        vp[:, t, :], hn[:, nt * 128:(nt + 1) * 128], wv[:],
                        start=True, stop=True,
                    )
                tsl = slice(h * 16 + r * 4, h * 16 + (r + 1) * 4)
                nc.vector.tensor_copy(vt[:, tsl, :], vp[:])

            emit_hn(0, "act")
            for r in range(2):
                emit_k_round(0, r)
            for r in range(2):
                emit_q_round(0, r)
            emit_hn(1, "dve")
            for r in range(4):
                emit_v_round(0, r)
            for r in range(2):
                emit_k_round(1, r)
            for r in range(4):
                emit_v_round(1, r)

            # ------------- softmax/projection tail emitters -------------
            # Reuse the S PSUM tags so the h0 tail interleaves into h1's
            # main loop with no pool barrier.
            def emit_den_front(ib):
                sl = slice(ib * IBLK, (ib + 1) * IBLK)
                rt = ps_s.tile([C, HQ], f32, tag=f"s{ib % 2}")
                rp = rt[0:1, 0:IBLK]
                nc.tensor.matmul(rp, ones_c[:], acc[:, sl], start=True, stop=True)
                st = stg.tile([1, IBLK], f32, tag="stage")
                nc.vector.tensor_copy(st[:], rp)
                # scatter [1, 512] -> 16 partitions x 32
                nc.sync.dma_start(rs128[ib * 16:(ib + 1) * 16, :], st[:])

            def emit_recip(pair):
                # DVE partition base must be 32-aligned: recip per ib-pair
                nc.vector.reciprocal(
                    rc128[pair * 32:(pair + 1) * 32, :],
                    rs128[pair * 32:(pair + 1) * 32, :],
                )

            def emit_den_back(ib):
                sl = slice(ib * IBLK, (ib + 1) * IBLK)
                nc.sync.dma_start(recip[:, sl], rc128[ib * 16:(ib + 1) * 16, :])
                bt = ps_s.tile([C, HQ], f32, tag=f"s{ib % 2}")
                bp = bt[:, 0:IBLK]
                pp = bt[:, IBLK:2 * IBLK]
                nc.tensor.matmul(bp, ones_r[:], recip[:, sl], start=True, stop=True)
                o_nrm = stg.tile([C, IBLK], bf16, tag="onrm")
                nc.vector.tensor_mul(o_nrm[:], o_sb[:, sl], bp)
                nc.tensor.matmul(pp, wp[:], o_nrm[:], start=True, stop=True)
                nc.vector.tensor_scalar(
                    out_sb[:, sl], pp, beff[:], None, mybir.AluOpType.add
                )
                nc.gpsimd.tensor_add(out_sb[:, sl], out_sb[:, sl], x_sb[:, sl])
                nc.scalar.dma_start(out_dram[:, sl], out_sb[:, sl])

            # ---------------- main attention loop ----------------
            # Software-pipelined: iteration j emits S(j) matmuls, exp(j),
            # then O(j-1), so the PE never stalls on its own iteration's exp.
            def tail_step(h, j):
                # interleave h0's tail into h1's loop, one op-group per slot
                if h != 1:
                    return
                if j in (1, 3, 5, 7):
                    emit_den_front(j // 2)
                elif j == 9:
                    emit_recip(0)
                elif j == 11:
                    emit_den_back(0)
                elif j == 13:
                    emit_den_back(1)
                elif j == 15:
                    emit_recip(1)
                elif j == 17:
                    emit_den_back(2)
                elif j == 19:
                    emit_den_back(3)

            for h in range(NHALF):
                hsl = slice(h * HWID, (h + 1) * HWID)
                if h == 1:
                    for r in range(2):
                        emit_q_round(1, r)
                op = ps_o.tile([C, HWID], f32, tag="o_ps")
                prev = None
                for j in range(NJ):
                    is_dve = (j % DVE_EXP) == DVE_EXP - 1
                    pt = ptpool.tile([C, HWID], bf16)
                    for half in range(2):
                        sp = ps_s.tile([C, HQ], f32, tag=f"s{half}")
                        for kk in range(2):
                            qoff = h * HWID + half * HQ + kk * IBLK
                            nc.tensor.matmul(
                                sp[:, kk * IBLK:(kk + 1) * IBLK],
                                k_r[:, j * 128:(j + 1) * 128],
                                q_r[:, qoff:qoff + IBLK],
                                start=True, stop=True,
                            )
                        psl = slice(half * HQ, (half + 1) * HQ)
                        if is_dve:
                            nc.vector.tensor_scalar(
                                pt[:, psl].bitcast(i16), sp[:], EXP_A, EXP_B,
                                mybir.AluOpType.mult, mybir.AluOpType.add,
                            )
                        else:
                            nc.scalar.activation(pt[:, psl], sp[:], AF.Exp)
                    if prev is not None:
                        for kk in range(4):
                            sl = slice(kk * IBLK, (kk + 1) * IBLK)
                            nc.tensor.matmul(
                                op[:, sl], vt[:, j - 1, :], prev[:, sl],
                                start=(j == 1), stop=False,
                            )
                    if j == 0:
                        nc.vector.tensor_copy(acc[:, hsl], pt[:])
                    else:
                        nc.vector.tensor_add(acc[:, hsl], acc[:, hsl], pt[:])
                    prev = pt
                    tail_step(h, j)
                for kk in range(4):
                    sl = slice(kk * IBLK, (kk + 1) * IBLK)
                    nc.tensor.matmul(
                        op[:, sl], vt[:, NJ - 1, :], prev[:, sl],
                        start=False, stop=True,
                    )
                nc.vector.tensor_copy(o_sb[:, hsl], op[:])

            # h1 tail (h0's was interleaved above)
            for ib in range(4, 8):
                emit_den_front(ib)
            emit_recip(2)
            emit_den_back(4)
            emit_den_back(5)
            emit_recip(3)
            emit_den_back(6)
            emit_den_back(7)

    nc.compile()
    return nc


def _get_nc():
    if "nc" not in _CACHE:
        _CACHE["nc"] = _build()
    return _CACHE["nc"]


def _prep_inputs(x, gamma, beta, w_qkv, b_qkv, w_proj, b_proj):
    x = np.ascontiguousarray(x, dtype=np.float32)
    w_qkv = np.asarray(w_qkv, dtype=np.float32)
    b_qkv = np.asarray(b_qkv, dtype=np.float32)
    w_proj = np.asarray(w_proj, dtype=np.float32)
    b_proj = np.asarray(b_proj, dtype=np.float32)

    wq = w_qkv[0:C, :]
    wk = w_qkv[C:2 * C, :]
    wv = w_qkv[2 * C:3 * C, :]
    bqv = b_qkv[0:C]
    bvv = b_qkv[2 * C:3 * C]

    wqT = np.ascontiguousarray((wq * SCALE).T)
    wkT = np.ascontiguousarray(wk.T)
    wvT = np.ascontiguousarray(wv.T)
    wpT = np.ascontiguousarray(w_proj.T)
    beff = (b_proj + w_proj @ bvv).astype(np.float32)

    ig = np.zeros((C, GROUPS), np.float32)
    ig[np.arange(C), np.arange(C) // GSIZE] = 1.0
    igt = np.ascontiguousarray(ig.T)

    common = {
        "gamma": np.asarray(gamma, np.float32).reshape(C, 1),
        "beta": np.asarray(beta, np.float32).reshape(C, 1),
        "bq": (bqv * SCALE).reshape(C, 1),
        "beff": beff.reshape(C, 1),
        "wqT": wqT,
        "wkT": wkT,
        "wvT": wvT,
        "wpT": wpT,
        "ig": ig,
        "igt": igt,
    }
    in_maps = []
    for b in range(B):
        m = dict(common)
        m["x"] = np.ascontiguousarray(x[b].reshape(C, HW))
        in_maps.append(m)
    return in_maps


def kernel(x, gamma, beta, w_qkv, b_qkv, w_proj, b_proj):
    from concourse.bass_utils import run_bass_kernel_spmd

    nc = _get_nc()
    in_maps = _prep_inputs(x, gamma, beta, w_qkv, b_qkv, w_proj, b_proj)
    res = run_bass_kernel_spmd(nc, in_maps, list(range(B)))
    out = np.stack([res.results[b]["out"] for b in range(B)], axis=0)
    return out.reshape(B, C, H, W).astype(np.float32)


# revision 5
# speedup vs baseline: 1.0196x; 1.0196x over previous
"""Trainium2 Bass kernel for nn_AttentionBlock (GroupNorm + 1x1-conv QKV +
full self-attention over N=HW=4096 + output projection + residual).

Distribution: data-parallel over batch B=8, one batch element per NeuronCore.

Per-core layout / algorithm (C=128 channels on SBUF partitions, N=4096 free):
  1. GroupNorm stats via two ACT passes (Square + Identity, both with
     accum_out row-sums), cross-partition group combine via tiny indicator
     matmuls on the PE.
  2. hn = a_c * x + b_c  (ACT + DVE, output bf16).
  3. Q, K in natural [c, n] layout (lhsT = host-pretransposed weights, bf16
     so the FWL fast-weight-load path engages); V^T in [n, c] tile-major
     layout (lhsT = hn tiles).
  4. Main loop (2 halves x 32 j-tiles), software-pipelined so the PE never
     waits on its own iteration's exp: emit S(j) -> exp(j) -> O(j-1).
     exp is split across engines: odd j-tiles' second half goes to DVE via
     the Schraudolph bit-trick in bf16 bit-space (1/4 of the work):
     p = bitcast_bf16(int16(A*s + B)), A = 2^7/ln2 (+-3.3% rel err on the
     attention weights, which post-softmax-averaging is ~1e-3 on the
     output).  Denominator partials in acc (bf16, DVE 2x adds), reduced by
     a ones^T matmul at the end.
  5. Tail (rowsum -> reciprocal -> broadcast -> O-normalize -> projection ->
     residual) for the first half is interleaved into the second half's
     main loop, reusing the S PSUM tags, so it hides under main-loop work.

Bias algebra: b_k is dropped entirely -- k_j = Wk hn_j + b_k adds q_i.b_k to
every logit of query i, a per-i constant that cancels exactly in softmax.
b_q folded into the Q PSUM->SBUF copy; b_v folded into
b_eff = b_proj + w_proj @ b_v (host precompute, exact).  The attention scale
C^-0.5 is folded into w_q/b_q on the host (exact reparameterization).
"""

import numpy as np

B, C, H, W = 8, 128, 64, 64
HW = H * W                      # 4096
GROUPS = 8
GSIZE = C // GROUPS             # 16
EPS = 1e-5
NJ = HW // 128                  # 32 j-tiles
IBLK = 512
NIB = HW // IBLK                # 8 i-blocks
NHALF = 2
HWID = HW // NHALF              # 2048
SCALE = float(C) ** -0.5
# Schraudolph fast-exp in bf16 bit-space: p = bitcast_bf16(int16(A*s + B));
# B tuned for truncating f32->i16 conversion, minimax rel err ~3.3e-2 on
# s in [-8, 8].  int16 IS the bf16 bit pattern, so DVE-exp'd tiles are
# bf16 like the ACT ones -- one uniform pipeline.
EXP_A = float((1 << 7)) / float(np.log(2.0))
EXP_B = float(127 * (1 << 7)) - 5.10

_CACHE = {}


def _build():
    from contextlib import ExitStack

    import concourse.bacc as bacc
    import concourse.tile as tile
    from concourse import mybir

    f32 = mybir.dt.float32
    bf16 = mybir.dt.bfloat16
    i16 = mybir.dt.int16
    AF = mybir.ActivationFunctionType

    nc = bacc.Bacc("TRN2", target_bir_lowering=False, debug=False)

    x_in = nc.dram_tensor("x", [C, HW], f32, kind="ExternalInput")
    gamma_in = nc.dram_tensor("gamma", [C, 1], f32, kind="ExternalInput")
    beta_in = nc.dram_tensor("beta", [C, 1], f32, kind="ExternalInput")
    bq_in = nc.dram_tensor("bq", [C, 1], f32, kind="ExternalInput")
    beff_in = nc.dram_tensor("beff", [C, 1], f32, kind="ExternalInput")
    wq_in = nc.dram_tensor("wqT", [C, C], f32, kind="ExternalInput")
    wk_in = nc.dram_tensor("wkT", [C, C], f32, kind="ExternalInput")
    wv_in = nc.dram_tensor("wvT", [C, C], f32, kind="ExternalInput")
    wp_in = nc.dram_tensor("wpT", [C, C], f32, kind="ExternalInput")
    ig_in = nc.dram_tensor("ig", [C, GROUPS], f32, kind="ExternalInput")
    igt_in = nc.dram_tensor("igt", [GROUPS, C], f32, kind="ExternalInput")
    out_dram = nc.dram_tensor("out", [C, HW], f32, kind="ExternalOutput")


    with tile.TileContext(nc) as tc, ExitStack() as ctx, \
         nc.allow_low_precision(reason="bf16 attention pipeline + fast-exp; "
                                "error budget audited vs the 2e-2 gate"):
        const = ctx.enter_context(tc.tile_pool(name="const", bufs=1))
        big = ctx.enter_context(tc.tile_pool(name="big", bufs=1))
        stats = ctx.enter_context(tc.tile_pool(name="stats", bufs=1))
        ptpool = ctx.enter_context(tc.tile_pool(name="pt", bufs=3))
        row = ctx.enter_context(tc.tile_pool(name="row", bufs=1))
        stg = ctx.enter_context(tc.tile_pool(name="stage", bufs=3))

        # ---------------- load x first (sync queue), consts on the ACT
        # HWDGE queue so they don't serialize behind/ahead of x -------------
        NCH = 4
        CHW = HW // NCH
        x_sb = big.tile([C, HW], f32, tag="x")
        for ch in range(NCH):
            sl = slice(ch * CHW, (ch + 1) * CHW)
            eng = nc.sync if ch % 2 == 0 else nc.scalar
            eng.dma_start(x_sb[:, sl], x_in[:, sl])

        def cload(t_in, shape, tag):
            t = const.tile(shape, f32, tag=tag)
            nc.sync.dma_start(t[:], t_in[:])
            return t

        gamma = cload(gamma_in, [C, 1], "c_gamma")
        beta = cload(beta_in, [C, 1], "c_beta")
        bq = cload(bq_in, [C, 1], "c_bq")
        beff = cload(beff_in, [C, 1], "c_beff")
        ig = cload(ig_in, [C, GROUPS], "c_ig")
        igt = cload(igt_in, [GROUPS, C], "c_igt")
        wq_f = cload(wq_in, [C, C], "c_wq_f")
        wk_f = cload(wk_in, [C, C], "c_wk_f")
        wv_f = cload(wv_in, [C, C], "c_wv_f")
        wp_f = cload(wp_in, [C, C], "c_wp_f")

        wq = const.tile([C, C], bf16)
        nc.vector.tensor_copy(wq[:], wq_f[:])
        wk = const.tile([C, C], bf16)
        nc.vector.tensor_copy(wk[:], wk_f[:])
        wv = const.tile([C, C], bf16)
        nc.vector.tensor_copy(wv[:], wv_f[:])
        wp = const.tile([C, C], bf16)
        nc.vector.tensor_copy(wp[:], wp_f[:])

        ones_c = const.tile([C, 1], bf16)
        nc.vector.memset(ones_c[:], 1.0)
        ones_r = const.tile([1, C], bf16)
        nc.vector.memset(ones_r[:], 1.0)
        eps_t = const.tile([GROUPS, 1], f32)
        nc.vector.memset(eps_t[:], EPS)
        magic_t = const.tile([GROUPS, 1], mybir.dt.uint32)
        nc.vector.memset(magic_t[:], 0x5F3759DF)
        c15_t = const.tile([GROUPS, 1], f32)
        nc.vector.memset(c15_t[:], 1.5)

        # ---------------- groupnorm stats (split across DVE and ACT) ----
        st2 = stats.tile([C, 2], f32)
        s2p = stats.tile([C, NCH], f32)
        s1p = stats.tile([C, NCH], f32)
        adum = stats.tile([C, CHW], f32)
        for ch in range(NCH):  # x^2 sums on ACT, x sums on DVE
            sl = slice(ch * CHW, (ch + 1) * CHW)
            nc.scalar.activation(
                adum[:], x_sb[:, sl], AF.Square, accum_out=s2p[:, ch:ch + 1]
            )
            nc.vector.reduce_sum(
                s1p[:, ch:ch + 1], x_sb[:, sl], axis=mybir.AxisListType.X
            )
        warm = stats.tile([GROUPS, 1], f32)
        nc.scalar.activation(warm[:], eps_t[:], AF.Exp)
        nc.vector.reduce_sum(st2[:, 1:2], s2p[:], axis=mybir.AxisListType.X)
        nc.vector.reduce_sum(st2[:, 0:1], s1p[:], axis=mybir.AxisListType.X)

        # PSUM layout for the whole kernel body: two S tiles (2 banks each,
        # independently released) + one O accumulator (4 banks). The QKV
        # rounds, groupnorm matmuls AND the softmax/projection tail all
        # borrow the S slots so there is no pool barrier anywhere.
        acc = big.tile([C, HW], bf16, tag="acc")
        o_sb = big.tile([C, HW], bf16, tag="o")
        recip = row.tile([1, HW], bf16)
        rs128 = stats.tile([C, HW // C], f32)  # [128, 32]
        rc128 = stats.tile([C, HW // C], bf16)
        out_sb = big.tile([C, HW], f32, tag="scratch")
        HQ = HWID // 2  # 1024
        with tc.tile_pool(name="ps_s", bufs=1, space="PSUM") as ps_s, \
             tc.tile_pool(name="ps_o", bufs=1, space="PSUM") as ps_o:
            gs_ps = ps_s.tile([GROUPS, 2], f32, tag="s0")
            nc.tensor.matmul(gs_ps[:], ig[:], st2[:], start=True, stop=True)
            gstats = stats.tile([GROUPS, 2], f32)
            nc.vector.tensor_copy(gstats[:], gs_ps[:])
            inv_n = 1.0 / float(GSIZE * HW)
            gmean = stats.tile([GROUPS, 1], f32)
            nc.vector.tensor_scalar_mul(gmean[:], gstats[:, 0:1], inv_n)
            gm2 = stats.tile([GROUPS, 1], f32)
            nc.vector.tensor_scalar_mul(gm2[:], gstats[:, 1:2], inv_n)
            gmsq = stats.tile([GROUPS, 1], f32)
            nc.vector.tensor_mul(gmsq[:], gmean[:], gmean[:])
            gvar = stats.tile([GROUPS, 1], f32)
            nc.vector.tensor_sub(gvar[:], gm2[:], gmsq[:])
            gve = stats.tile([GROUPS, 1], f32)
            nc.vector.tensor_scalar(
                gve[:], gvar[:], eps_t[:], None, mybir.AluOpType.add
            )
            # rstd = rsqrt(var+eps): quake initial guess + Newton steps (DVE
            # only -- ACT Sqrt/Ln would each force a ~1.3us table-set swap)
            u32 = mybir.dt.uint32
            gu = stats.tile([GROUPS, 1], u32)
            nc.vector.tensor_scalar(
                gu[:], gve[:].bitcast(u32), 1, None,
                mybir.AluOpType.logical_shift_right,
            )
            nc.vector.tensor_sub(gu[:], magic_t[:], gu[:])
            gy = stats.tile([GROUPS, 1], f32)
            nc.vector.tensor_copy(gy[:], gu[:].bitcast(f32))
            gh = stats.tile([GROUPS, 1], f32)
            nc.vector.tensor_scalar_mul(gh[:], gve[:], 0.5)
            gt = stats.tile([GROUPS, 1], f32)
            for _ in range(2):
                nc.vector.tensor_mul(gt[:], gy[:], gy[:])
                nc.vector.tensor_mul(gt[:], gt[:], gh[:])
                nc.vector.tensor_sub(gt[:], c15_t[:], gt[:])
                nc.vector.tensor_mul(gy[:], gy[:], gt[:])
            gmr = stats.tile([GROUPS, 2], f32)
            nc.vector.tensor_copy(gmr[:, 1:2], gy[:])
            nc.vector.tensor_copy(gmr[:, 0:1], gmean[:])

            bc_ps = ps_s.tile([C, 2], f32, tag="s1")
            nc.tensor.matmul(bc_ps[:], igt[:], gmr[:], start=True, stop=True)
            a_c = stats.tile([C, 1], f32)
            b_c = stats.tile([C, 1], f32)
            tmc = stats.tile([C, 1], f32)
            nc.vector.tensor_scalar_mul(a_c[:], gamma[:], bc_ps[:, 1:2])
            nc.vector.tensor_scalar_mul(tmc[:], a_c[:], bc_ps[:, 0:1])
            nc.vector.tensor_sub(b_c[:], beta[:], tmc[:])

            hn = big.tile([C, HW], bf16, tag="hn")
            q_r = big.tile([C, HW], bf16, tag="q")
            k_r = big.tile([C, HW], bf16, tag="k")
            vt = big.tile([C, NJ, C], bf16, tag="vt")

            def emit_hn(h, engine):
                hs = slice(h * HWID, (h + 1) * HWID)
                if engine == "act":
                    nc.scalar.activation(
                        hn[:, hs], x_sb[:, hs], AF.Identity, bias=b_c[:], scale=a_c[:]
                    )
                else:
                    nc.vector.tensor_scalar(
                        hn[:, hs], x_sb[:, hs], a_c[:], b_c[:],
                        mybir.AluOpType.mult, mybir.AluOpType.add,
                    )

            def emit_k_round(h, r):  # r in 0..1, [C, HQ] rounds
                kp = ps_s.tile([C, HQ], f32, tag=f"s{r % 2}")
                for kk in range(2):
                    off = h * HWID + r * HQ + kk * IBLK
                    nc.tensor.matmul(
                        kp[:, kk * IBLK:(kk + 1) * IBLK], wk[:],
                        hn[:, off:off + IBLK], start=True, stop=True,
                    )
                # b_k dropped: a per-query constant in the logits, cancels in
                # softmax exactly.
                nc.vector.tensor_copy(
                    k_r[:, h * HWID + r * HQ:h * HWID + (r + 1) * HQ], kp[:]
                )

            def emit_q_round(h, r):
                qp = ps_s.tile([C, HQ], f32, tag=f"s{r % 2}")
                for kk in range(2):
                    off = h * HWID + r * HQ + kk * IBLK
                    nc.tensor.matmul(
                        qp[:, kk * IBLK:(kk + 1) * IBLK], wq[:],
                        hn[:, off:off + IBLK], start=True, stop=True,
                    )
                nc.vector.tensor_scalar(
                    q_r[:, h * HWID + r * HQ:h * HWID + (r + 1) * HQ], qp[:],
                    bq[:], None, mybir.AluOpType.add,
                )

            def emit_v_round(h, r):  # r in 0..3, 4 n-tiles per round
                vp = ps_s.tile([C, 4, C], f32, tag=f"s{r % 2}")
                for t in range(4):
                    nt = h * 16 + r * 4 + t
                    nc.tensor.matmul(
                        vp[:, t, :], hn[:, nt * 128:(nt + 1) * 128], wv[:],
                        start=True, stop=True,
                    )
                tsl = slice(h * 16 + r * 4, h * 16 + (r + 1) * 4)
                nc.vector.tensor_copy(vt[:, tsl, :], vp[:])

            emit_hn(0, "act")
            for r in range(2):
                emit_k_round(0, r)
            for r in range(2):
                emit_q_round(0, r)
            emit_hn(1, "dve")
            for r in range(4):
                emit_v_round(0, r)
            for r in range(2):
                emit_k_round(1, r)
            for r in range(4):
                emit_v_round(1, r)

            # ------------- softmax/projection tail emitters -------------
            # Reuse the S PSUM tags so the h0 tail interleaves into h1's
            # main loop with no pool barrier.
            def emit_den_front(ib):
                sl = slice(ib * IBLK, (ib + 1) * IBLK)
                rt = ps_s.tile([C, HQ], f32, tag=f"s{ib % 2}")
                rp = rt[0:1, 0:IBLK]
                nc.tensor.matmul(rp, ones_c[:], acc[:, sl], start=True, stop=True)
                st = stg.tile([1, IBLK], f32, tag="stage")
                nc.vector.tensor_copy(st[:], rp)
                # scatter [1, 512] -> 16 partitions x 32
                nc.sync.dma_start(rs128[ib * 16:(ib + 1) * 16, :], st[:])

            def emit_recip(pair):
                # DVE partition base must be 32-aligned: recip per ib-pair
                nc.vector.reciprocal(
                    rc128[pair * 32:(pair + 1) * 32, :],
                    rs128[pair * 32:(pair + 1) * 32, :],
                )

            def emit_den_back(ib):
                sl = slice(ib * IBLK, (ib + 1) * IBLK)
                nc.sync.dma_start(recip[:, sl], rc128[ib * 16:(ib + 1) * 16, :])
                bt = ps_s.tile([C, HQ], f32, tag=f"s{ib % 2}")
                bp = bt[:, 0:IBLK]
                pp = bt[:, IBLK:2 * IBLK]
                nc.tensor.matmul(bp, ones_r[:], recip[:, sl], start=True, stop=True)
                o_nrm = stg.tile([C, IBLK], bf16, tag="onrm")
                nc.vector.tensor_mul(o_nrm[:], o_sb[:, sl], bp)
                nc.tensor.matmul(pp, wp[:], o_nrm[:], start=True, stop=True)
                nc.vector.tensor_scalar(
                    out_sb[:, sl], pp, beff[:], None, mybir.AluOpType.add
                )
                # residual add: GPSIMD for the blocks hidden under the main
                # loop, DVE for the latency-critical final blocks
                if ib < 4:
                    nc.gpsimd.tensor_add(out_sb[:, sl], out_sb[:, sl], x_sb[:, sl])
                else:
                    nc.vector.tensor_add(out_sb[:, sl], out_sb[:, sl], x_sb[:, sl])
                nc.scalar.dma_start(out_dram[:, sl], out_sb[:, sl])

            # ---------------- main attention loop ----------------
            # Software-pipelined: iteration j emits S(j) matmuls, exp(j),
            # then O(j-1), so the PE never stalls on its own iteration's exp.
            def tail_step(h, j):
                # interleave h0's tail into h1's loop, one op-group per slot
                if h != 1:
                    return
                if j in (1, 3, 5, 7):
                    emit_den_front(j // 2)
                elif j == 9:
                    emit_recip(0)
                elif j == 11:
                    emit_den_back(0)
                elif j == 13:
                    emit_den_back(1)
                elif j == 15:
                    emit_recip(1)
                elif j == 17:
                    emit_den_back(2)
                elif j == 19:
                    emit_den_back(3)

            for h in range(NHALF):
                hsl = slice(h * HWID, (h + 1) * HWID)
                if h == 1:
                    for r in range(2):
                        emit_q_round(1, r)
                op = ps_o.tile([C, HWID], f32, tag="o_ps")
                prev = None
                for j in range(NJ):
                    pt = ptpool.tile([C, HWID], bf16)
                    for half in range(2):
                        sp = ps_s.tile([C, HQ], f32, tag=f"s{half}")
                        for kk in range(2):
                            qoff = h * HWID + half * HQ + kk * IBLK
                            nc.tensor.matmul(
                                sp[:, kk * IBLK:(kk + 1) * IBLK],
                                k_r[:, j * 128:(j + 1) * 128],
                                q_r[:, qoff:qoff + IBLK],
                                start=True, stop=True,
                            )
                        psl = slice(half * HQ, (half + 1) * HQ)
                        # every odd j: the second half's exp runs on DVE
                        # (Schraudolph, bf16 bit-space) concurrently with
                        # ACT's first half -- fine-grained engine split that
                        # keeps the 2-deep S pipeline from stalling.
                        if half == 1 and (j % 2) == 1:
                            nc.vector.tensor_scalar(
                                pt[:, psl].bitcast(i16), sp[:], EXP_A, EXP_B,
                                mybir.AluOpType.mult, mybir.AluOpType.add,
                            )
                        else:
                            nc.scalar.activation(pt[:, psl], sp[:], AF.Exp)
                    if prev is not None:
                        for kk in range(4):
                            sl = slice(kk * IBLK, (kk + 1) * IBLK)
                            nc.tensor.matmul(
                                op[:, sl], vt[:, j - 1, :], prev[:, sl],
                                start=(j == 1), stop=False,
                            )
                    if j == 0:
                        nc.vector.tensor_copy(acc[:, hsl], pt[:])
                    else:
                        nc.vector.tensor_add(acc[:, hsl], acc[:, hsl], pt[:])
                    prev = pt
                    tail_step(h, j)
                for kk in range(4):
                    sl = slice(kk * IBLK, (kk + 1) * IBLK)
                    nc.tensor.matmul(
                        op[:, sl], vt[:, NJ - 1, :], prev[:, sl],
                        start=False, stop=True,
                    )
                nc.vector.tensor_copy(o_sb[:, hsl], op[:])

            # h1 tail (h0's was interleaved above)
            for ib in range(4, 8):
                emit_den_front(ib)
            emit_recip(2)
            emit_den_back(4)
            emit_den_back(5)
            emit_recip(3)
            emit_den_back(6)
            emit_den_back(7)

    nc.compile()
    return nc


def _get_nc():
    if "nc" not in _CACHE:
        _CACHE["nc"] = _build()
    return _CACHE["nc"]


def _prep_inputs(x, gamma, beta, w_qkv, b_qkv, w_proj, b_proj):
    x = np.ascontiguousarray(x, dtype=np.float32)
    w_qkv = np.asarray(w_qkv, dtype=np.float32)
    b_qkv = np.asarray(b_qkv, dtype=np.float32)
    w_proj = np.asarray(w_proj, dtype=np.float32)
    b_proj = np.asarray(b_proj, dtype=np.float32)

    wq = w_qkv[0:C, :]
    wk = w_qkv[C:2 * C, :]
    wv = w_qkv[2 * C:3 * C, :]
    bqv = b_qkv[0:C]
    bvv = b_qkv[2 * C:3 * C]

    wqT = np.ascontiguousarray((wq * SCALE).T)
    wkT = np.ascontiguousarray(wk.T)
    wvT = np.ascontiguousarray(wv.T)
    wpT = np.ascontiguousarray(w_proj.T)
    beff = (b_proj + w_proj @ bvv).astype(np.float32)

    ig = np.zeros((C, GROUPS), np.float32)
    ig[np.arange(C), np.arange(C) // GSIZE] = 1.0
    igt = np.ascontiguousarray(ig.T)

    common = {
        "gamma": np.asarray(gamma, np.float32).reshape(C, 1),
        "beta": np.asarray(beta, np.float32).reshape(C, 1),
        "bq": (bqv * SCALE).reshape(C, 1),
        "beff": beff.reshape(C, 1),
        "wqT": wqT,
        "wkT": wkT,
        "wvT": wvT,
        "wpT": wpT,
        "ig": ig,
        "igt": igt,
    }
    in_maps = []
    for b in range(B):
        m = dict(common)
        m["x"] = np.ascontiguousarray(x[b].reshape(C, HW))
        in_maps.append(m)
    return in_maps


def kernel(x, gamma, beta, w_qkv, b_qkv, w_proj, b_proj):
    from concourse.bass_utils import run_bass_kernel_spmd

    nc = _get_nc()
    in_maps = _prep_inputs(x, gamma, beta, w_qkv, b_qkv, w_proj, b_proj)
    res = run_bass_kernel_spmd(nc, in_maps, list(range(B)))
    out = np.stack([res.results[b]["out"] for b in range(B)], axis=0)
    return out.reshape(B, C, H, W).astype(np.float32)
